# revision 9
# baseline (speedup 1.0000x reference)
"""Trainium2 Bass kernel for nn_Mixture_Loss_74053826118054.

Strategy (data parallel over batch B=256, 32 batches per core):
  Every term of the loss depends only on 5 per-(s,b)-row reductions over D:
    ll = sum_d l^2,  tt = sum_d t^2,  lt = sum_d l*t,
    ln = sum_d l[s]*l[s+1]  (consecutive sentences, same batch),
    tn = sum_d t[s]*t[s+1]
  (masked MSE = sum over valid rows of ll - 2lt + tt; cosines = dots/norms).

  Layout: rows are batch-major (b, s). Each SBUF partition holds a window of
  9 consecutive rows (8 + 1 overlap), so consecutive-row products are
  free-axis slices — no partition shifts (illegal on compute engines).
  l and t are stacked into one DRAM tensor and each 1024-column chunk is
  fetched with a single DMA (so every compute op needs at most one new
  semaphore: TPB instructions only encode one sync wait). Per chunk j:
  ACT computes both squares with fused accumulate, GpSimd computes l*t with
  fused accumulate, DVE computes the two shifted products with fused
  tensor_tensor_reduce. Fused-op full-size `out` operands are tiny (128,1)
  dummies broadcast to shape; each op gets a unique dummy so no WAW sems
  appear. The overlap row costs +12.5% DMA on a memory-bound kernel.

  The tiny O(S*B) finish (cos, deltas, compaction, delta-of-delta) runs on
  host in float64.
"""

import numpy as np

import concourse.bass as bass
import concourse.mybir as mybir
from concourse import tile
from concourse.bass_utils import run_bass_kernel_spmd

F32 = mybir.dt.float32
AF = mybir.ActivationFunctionType
ALU = mybir.AluOpType

N_CORES = 8
S, B, D = 64, 256, 1024
B_SHARD = B // N_CORES          # 32 batches per core
ROWS = B_SHARD * S              # 2048 real rows per core
G = 8                           # rows per window
P = 128                         # partitions per tile
NMEGA = ROWS // (G * P)         # 2 window-sets per core
ROWS_PAD = (P * NMEGA + 1) * G  # 2056: one extra window of padding rows
NCOL = NMEGA * G                # 16 result columns
QUANTS = ("ll", "tt", "lt", "ln", "tn")

_cached_nc = None


def _build_program():
    global _cached_nc
    if _cached_nc is not None:
        return _cached_nc

    nc = bass.Bass()
    x_in = nc.dram_tensor("x", [2, ROWS_PAD, D], F32, kind="ExternalInput")
    outs = {q: nc.dram_tensor(q, [P, NCOL], F32, kind="ExternalOutput")
            for q in QUANTS}

    # (half, rows, d) -> (window, half, slot, d) so a (128, 2, 1024) chunk is
    # one strided DMA: partition = window, middle dim = l/t half.
    x_v = x_in.rearrange("h (w g) d -> w h g d", g=G)   # (257, 2, 8, 1024)

    with tile.TileContext(nc) as tc:
        with tc.tile_pool(name="inp", bufs=6) as inp, \
             tc.tile_pool(name="scr", bufs=1) as scr, \
             tc.tile_pool(name="res", bufs=1) as res:
            rt = {q: res.tile([P, NCOL], F32, tag=q, name=f"rt_{q}")
                  for q in QUANTS}

            for i in range(NMEGA):
                w0 = i * P
                xt = []
                for j in range(G + 1):
                    t_ = inp.tile([P, 2 * D], F32, tag="x", name=f"x_{i}_{j}")
                    if j < G:
                        nc.sync.dma_start(out=t_[:], in_=x_v[w0:w0 + P, :, j, :])
                    else:  # overlap row: next window's slot 0
                        nc.sync.dma_start(out=t_[:],
                                          in_=x_v[w0 + 1:w0 + P + 1, :, 0, :])
                    xt.append(t_)

                # pre-observe chunk 0 on DVE so the first TTR needs one sem
                tiny = scr.tile([P, 1], F32, tag=f"tiny{i}", name=f"tiny_{i}")
                nc.vector.tensor_copy(tiny[:], xt[0][:, 0:1])

                for j in range(G):
                    col = i * G + j
                    lc = xt[j][:, 0:D]
                    tc2 = xt[j][:, D:2 * D]
                    ln_ = xt[j + 1][:, 0:D]
                    tn_ = xt[j + 1][:, D:2 * D]

                    def dummy(kind):
                        return scr.tile([P, 1], F32, tag=f"{kind}{i}_{j}",
                                        name=f"{kind}_{i}_{j}")

                    nc.scalar.activation(
                        dummy("da").broadcast_to(lc.shape), lc, AF.Square,
                        accum_out=rt["ll"][:, col:col + 1])
                    nc.scalar.activation(
                        dummy("db").broadcast_to(lc.shape), tc2, AF.Square,
                        accum_out=rt["tt"][:, col:col + 1])

                    # Pool lacks scalar_tensor_tensor in this walrus: plain
                    # multiply on Pool, then ACT Copy-accumulate reduces it.
                    sp = scr.tile([P, D], F32, tag="sp", bufs=3,
                                  name=f"sp_{i}_{j}")
                    nc.gpsimd.tensor_tensor(out=sp[:], in0=lc, in1=tc2,
                                            op=ALU.mult)
                    nc.scalar.activation(
                        dummy("dp").broadcast_to(lc.shape), sp[:], AF.Copy,
                        accum_out=rt["lt"][:, col:col + 1])

                    nc.vector.scalar_tensor_tensor(
                        out=dummy("dv").broadcast_to(lc.shape),
                        in0=lc, scalar=0.0, in1=ln_,
                        op0=ALU.bypass, op1=ALU.mult,
                        accum_out=rt["ln"][:, col:col + 1])
                    nc.vector.scalar_tensor_tensor(
                        out=dummy("dw").broadcast_to(lc.shape),
                        in0=tc2, scalar=0.0, in1=tn_,
                        op0=ALU.bypass, op1=ALU.mult,
                        accum_out=rt["tn"][:, col:col + 1])

            for q in QUANTS:
                nc.sync.dma_start(out=outs[q][:], in_=rt[q][:])

    _legalize_waits(nc)
    _cached_nc = nc
    return nc


def _legalize_waits(nc):
    """Walrus encodes at most one sync wait per TPB instruction. Split any
    non-DMA instruction carrying N>1 waits into N-1 preceding same-engine
    EventSemaphore waits plus the instruction keeping one wait."""
    dummy_sem = nc.alloc_semaphore("legalize_pad")
    cur_insts = nc.cur_bb.bb.instructions
    for bb in nc.main_func.blocks:
        insts = bb.instructions
        new_list = []
        changed = False
        for ins in insts:
            si = ins.sync_info
            waits = list(si.on_wait) if si is not None and si.on_wait else []
            if len(waits) > 1:
                for w in waits[:-1]:
                    ev = nc.engines[ins.engine].wait_ge(dummy_sem, 0).ins
                    # wait_ge appends to the current block; reclaim it
                    popped = cur_insts.pop()
                    assert popped is ev
                    ev.sync_info.on_wait = [w]
                    new_list.append(ev)
                si.on_wait = [waits[-1]]
                changed = True
            new_list.append(ins)
        if changed:
            insts[:] = new_list


def _unpack(arr):
    """(128, NCOL) device layout -> (B_SHARD, S): row r = i*1024 + p*8 + j."""
    return (arr.reshape(P, NMEGA, G).transpose(1, 0, 2)
            .reshape(ROWS).reshape(B_SHARD, S))


def _run_device(logits, tgt_out, trace=False):
    """Returns dict q -> (B, S) float32 row-dot arrays, plus kernel results."""
    nc = _build_program()
    # (S, B, D) -> (B, S, D) batch-major, split over cores along B
    lb = np.ascontiguousarray(np.swapaxes(logits, 0, 1))
    tb = np.ascontiguousarray(np.swapaxes(tgt_out, 0, 1))
    in_maps = []
    for c in range(N_CORES):
        sl = slice(c * B_SHARD, (c + 1) * B_SHARD)
        x = np.zeros((2, ROWS_PAD, D), np.float32)
        x[0, :ROWS] = lb[sl].reshape(ROWS, D)
        x[1, :ROWS] = tb[sl].reshape(ROWS, D)
        in_maps.append({"x": x})
    kres = run_bass_kernel_spmd(nc, in_maps, list(range(N_CORES)), trace=trace)
    full = {}
    for q in QUANTS:
        full[q] = np.concatenate(
            [_unpack(kres.results[c][q]) for c in range(N_CORES)], axis=0)
    return full, kres


def _finish_host(rows, mask):
    """Host-side float64 finish: reproduce reference semantics exactly."""
    ll = rows["ll"].astype(np.float64)
    tt = rows["tt"].astype(np.float64)
    lt = rows["lt"].astype(np.float64)
    ln = rows["ln"].astype(np.float64)
    tn = rows["tn"].astype(np.float64)

    valid = ~mask                     # (B, S)
    n_valid = float(valid.sum())

    # masked MSE: sum over valid rows of sum_d (l-t)^2 = ll - 2lt + tt
    mse = ((ll - 2.0 * lt + tt) * valid).sum() / (n_valid * D)

    # CosineEmbeddingLoss part (eps = 1e-8)
    na = np.maximum(np.sqrt(ll), 1e-8)
    nb = np.maximum(np.sqrt(tt), 1e-8)
    c = lt / (na * nb)
    loss_cos = ((1.0 - c) * valid).sum() / n_valid

    # consecutive-sentence cosine deltas (eps = 1e-6), shape (B, S-1)
    nl = np.maximum(np.sqrt(ll), 1e-6)
    nt = np.maximum(np.sqrt(tt), 1e-6)
    d_l = ln[:, :S - 1] / (nl[:, :-1] * nl[:, 1:])
    d_t = tn[:, :S - 1] / (nt[:, :-1] * nt[:, 1:])
    pair_valid = valid[:, :-1] & valid[:, 1:]
    cnt = int(pair_valid.sum())
    loss_delta = (np.square(d_l - d_t) * pair_valid).sum() / max(cnt, 1)

    # delta-of-delta on the compacted (valid-only, batch-major) delta lists
    L = B * (S - 1)
    pvf = pair_valid.reshape(-1)

    def dd(d_flat):
        dense = np.zeros(L, np.float64)
        dense[:cnt] = d_flat[pvf]
        prev = dense[:-1]
        den = np.where(prev != 0, prev, 1e-6)
        return (dense[1:] - prev) / den

    dd_l = dd(d_l.reshape(-1))
    dd_t = dd(d_t.reshape(-1))
    dd_valid = np.arange(L - 1) < (cnt - 1)
    n_dd = float(max(cnt - 1, 1))
    loss_dd = (np.square(dd_l - dd_t) * dd_valid).sum() / n_dd / 100.0

    return mse + loss_cos + loss_delta + loss_dd


def kernel(logits, tgt_out, tgt_padding_mask, _trace=False):
    logits = np.asarray(logits, dtype=np.float32)
    tgt_out = np.asarray(tgt_out, dtype=np.float32)
    mask = np.asarray(tgt_padding_mask).astype(bool)
    rows, kres = _run_device(logits, tgt_out, trace=_trace)
    total = _finish_host(rows, mask)
    out = np.array(total, dtype=np.float32)
    if _trace:
        return out, kres
    return out
